# revision 1
# baseline (speedup 1.0000x reference)
"""Trainium2 Bass kernel for nn_CDFLearnableActivation (histogram binning).

Computes y = scale * cdf_table[clip(searchsorted(sorted_values,
round(x*100)/100, side='right'), 0, K-1)] over x (16, 4096, 2048) fp32,
data-parallel across 8 NeuronCores (x sharded on the leading dim; the tiny
tables are folded on the host and replicated per core).

Host folding: the whole (sorted_values, cdf_table, scale) pipeline collapses
into one 4096-entry table T indexed by j = clip(round_half_even(fl(100*x))
+ 2048, 0, 4095); T[m+2048] = fl(scale * cdf_table[clip(searchsorted(
sorted_values, fl(m/100), 'right'), 0, K-1)]) reproduces the reference
semantics exactly for every reachable fp32 x (tails saturate identically).

Device algorithm (TRN2 has no fast per-element gather -- stock GPSIMD
ap_gather measures ~33 Q7 cycles/index = ~465 ms/core -- so the lookup is
emulated exactly on the TensorEngine as a one-hot factorization):
  j is computed with exact fp32 arithmetic (RNE integer snap via +1.5*2^23,
  exact floor j>>6 via a biased snap); a = j>>6, b = j&63.
  Each 128x512 x-tile is PE-transposed so elements lie along the free dim.
  Per pair of transposed rows, with selection/reduction matrices as
  compile-time-constant stationaries (host inputs):
    B1 = statb_k.T @ bf16(b)   PE bf16: broadcast b to partitions (2 chunks)
    FB = (B1 == iota64)        DVE is_equal: exact one-hot of b (bf16 0/1)
    z  = sum_i Ci.T @ FB       PE: 3 accumulating bf16 matmuls against an
                               exact 3-way bf16 split of the block-diagonal
                               table matrix [[C,0],[0,C]], C[beta,alpha] =
                               T[64*alpha+beta]; PSUM fp32 restores T exactly
    A1 = statb_k.T @ bf16(a)   PE bf16 broadcast of a
    P  = (A1 == iota64) * zS   DVE fused is_equal+mult (zS = ACT copy of z)
    Y += statr_k.T @ P         PE fp32: masked column sums land on rows
                               2k/2k+1; zeros elsewhere; accumulated in PSUM
  The pair stages are emitted as a 4-deep software pipeline so PE/DVE/ACT
  stream concurrently; results are PE-transposed back and DMA'd out.

Every arithmetic step is bit-exact in fp32: verified bitwise-equal against
the jax reference on the full 134M-element input. Measured device time
~37 ms/core (pair loop ~30 ms, measured by in-kernel repetition deltas);
memory roofline is ~0.37 ms -- the gap is the price of emulating a gather
with dense TensorEngine work (~2 PE columns + ~2 DVE passes per element
pair-slot).
"""

import sys
sys.path.insert(0, "/opt/trn_rl_repo")

import numpy as np

M_TBL = 4096
J0 = 2048
SNAP = 12582912.0        # 1.5 * 2^23
FLOOR_BIAS = -0.4921875  # -(63/128)
F = 512
N_CORES = 8

_COMPILED = {}
_COMPILED_T = {}


def _build_device_table(sorted_values, cdf_table, scale):
    K = sorted_values.shape[0]
    m = np.arange(-J0, M_TBL - J0, dtype=np.float32)
    rounded = (m / np.float32(100.0)).astype(np.float32)
    idx = np.searchsorted(sorted_values.astype(np.float32), rounded, side="right")
    idx = np.clip(idx, 0, K - 1)
    return (np.float32(scale) * cdf_table.astype(np.float32)[idx]).astype(np.float32)


def _consts(tbl):
    cmat = tbl.reshape(64, 64).T  # cmat[beta, alpha] = tbl[64*alpha + beta]
    cmatbd = np.zeros((128, 128), dtype=np.float32)
    cmatbd[0:64, 0:64] = cmat
    cmatbd[64:128, 64:128] = cmat
    statb = np.zeros((128, 64 * 128), dtype=np.float32)
    statr = np.zeros((128, 64 * 128), dtype=np.float32)
    for k in range(64):
        statb[2 * k, 128 * k + 0:128 * k + 64] = 1.0
        statb[2 * k + 1, 128 * k + 64:128 * k + 128] = 1.0
        statr[0:64, 128 * k + 2 * k] = 1.0
        statr[64:128, 128 * k + 2 * k + 1] = 1.0
    iotap = (np.arange(128, dtype=np.float32) % 64).reshape(128, 1)
    eye = np.eye(128, dtype=np.float32)
    import jax.numpy as jnp

    def to16(a):
        return np.asarray(jnp.asarray(a).astype(jnp.bfloat16))

    statb16 = to16(statb)
    chi = to16(cmatbd)
    r1 = cmatbd - np.asarray(jnp.asarray(chi).astype(jnp.float32))
    cmid = to16(r1)
    r2 = r1 - np.asarray(jnp.asarray(cmid).astype(jnp.float32))
    clo = to16(r2)
    # exactness of the 3-way split, evaluated in fp32 left-to-right
    rec = (np.asarray(jnp.asarray(chi).astype(jnp.float32)) +
           np.asarray(jnp.asarray(cmid).astype(jnp.float32)))
    rec = rec.astype(np.float32) + np.asarray(jnp.asarray(clo).astype(jnp.float32))
    assert np.array_equal(rec.astype(np.float32), cmatbd), "bf16 split not exact"
    return (chi, cmid, clo), statb16, statr, iotap, eye


DUP = set()


def _emit(nc, tc, xap, yap, cols, consts_aps, n_unroll=2, pair_reps=1):
    """Emit the full per-core pipeline reading xap -> writing yap."""
    from concourse import bass, mybir

    f32 = mybir.dt.float32
    Alu = mybir.AluOpType
    n_tiles = cols // F
    (chi_ap, cmid_ap, clo_ap), statb, statr, iotap, eye = consts_aps

    with tc.tile_pool(name="const", bufs=1) as cpool:
        bf16d = mybir.dt.bfloat16
        chi_t = cpool.tile([128, 128], bf16d)
        nc.sync.dma_start(out=chi_t[:, :], in_=chi_ap[:, :])
        cmid_t = cpool.tile([128, 128], bf16d)
        nc.sync.dma_start(out=cmid_t[:, :], in_=cmid_ap[:, :])
        clo_t = cpool.tile([128, 128], bf16d)
        nc.sync.dma_start(out=clo_t[:, :], in_=clo_ap[:, :])
        bf16 = mybir.dt.bfloat16
        statb_t = cpool.tile([128, 64 * 128], bf16)
        nc.sync.dma_start(out=statb_t[:, :], in_=statb[:, :])
        statr_t = cpool.tile([128, 64 * 128], f32)
        nc.sync.dma_start(out=statr_t[:, :], in_=statr[:, :])
        iota_t = cpool.tile([128, 1], f32)
        nc.sync.dma_start(out=iota_t[:, :], in_=iotap[:, :])
        eye_t = cpool.tile([128, 128], f32)
        nc.sync.dma_start(out=eye_t[:, :], in_=eye[:, :])

        with (
            tc.tile_pool(name="sb", bufs=3) as sb,
            tc.tile_pool(name="psA", bufs=3, space="PSUM") as psA,
            tc.tile_pool(name="psB", bufs=1, space="PSUM") as psB,
        ):
            def body(t):
                xt = sb.tile([128, F], f32, tag="xt")
                nc.sync.dma_start(out=xt[:, :], in_=xap[:, bass.ts(t, F)])
                xT = psB.tile([128, F], f32, tag="xT")
                for q in range(4):
                    nc.tensor.transpose(
                        xT[:, 128 * q:128 * (q + 1)],
                        xt[:, 128 * q:128 * (q + 1)], eye_t[:, :])
                u = sb.tile([128, F], f32, tag="u")
                nc.vector.tensor_scalar(u[:, :], xT[:, :], 100.0, SNAP,
                                        Alu.mult, Alu.add)
                m0 = sb.tile([128, F], f32, tag="m0")
                nc.vector.tensor_scalar(m0[:, :], u[:, :], SNAP - J0, 0.0,
                                        Alu.subtract, Alu.max)
                j = sb.tile([128, F], f32, tag="j")
                nc.vector.tensor_scalar(j[:, :], m0[:, :],
                                        float(M_TBL - 1), None, Alu.min)
                w = sb.tile([128, F], f32, tag="w")
                nc.vector.tensor_scalar(w[:, :], j[:, :], 0.015625,
                                        FLOOR_BIAS, Alu.mult, Alu.add)
                af = sb.tile([128, F], f32, tag="af")
                nc.vector.tensor_scalar(af[:, :], w[:, :], SNAP, SNAP,
                                        Alu.add, Alu.subtract)
                bf = sb.tile([128, F], f32, tag="bf")
                nc.vector.scalar_tensor_tensor(bf[:, :], af[:, :], -64.0,
                                               j[:, :], Alu.mult, Alu.add)
                af16 = sb.tile([128, F], mybir.dt.bfloat16, tag="af16")
                nc.vector.tensor_copy(af16[:, :], af[:, :])
                bf16t = sb.tile([128, F], mybir.dt.bfloat16, tag="bf16t")
                nc.vector.tensor_copy(bf16t[:, :], bf[:, :])
                Yt = psB.tile([128, F], f32, tag="Yt")
                for _rep in range(pair_reps):
                  # 4-deep software pipeline over pairs; every cross-engine
                  # dependency crosses a slot boundary so all engines stream:
                  #   slot s: PE B1(s), A1(s-2), z(s-1), Yt(s-4)
                  #           DVE FB(s), P(s-3);  ACT zS(s-2)
                  A1s, FBs, zs, zSs, Ps = {}, {}, {}, {}, {}
                  for slot in range(68):
                      if slot < 64:
                          kk = slot
                          sb_k = statb_t[:, 128 * kk:128 * (kk + 1)]
                          B1 = psA.tile([128, F], f32, tag="Bz")
                          nc.tensor.matmul(B1[:, :], sb_k, bf16t[:, :],
                                           start=True, stop=True)
                          if 'B1' in DUP:
                              nc.tensor.matmul(B1[:, :], sb_k, bf16t[:, :],
                                               start=True, stop=True)
                          FB = sb.tile([128, F], mybir.dt.bfloat16, tag="FB")
                          nc.vector.tensor_scalar(FB[:, :], B1[:, :],
                                                  iota_t[:, :], None,
                                                  Alu.is_equal)
                          if 'FB' in DUP:
                              nc.vector.tensor_scalar(FB[:, :], B1[:, :],
                                                      iota_t[:, :], None,
                                                      Alu.is_equal)
                          FBs[kk] = FB
                      if 1 <= slot <= 64:
                          kk = slot - 1
                          z = psA.tile([128, F], f32, tag="Bz")
                          _fb = FBs.pop(kk)
                          nc.tensor.matmul(z[:, :], chi_t[:, :], _fb[:, :],
                                           start=True, stop=False,
                                           skip_group_check=True)
                          nc.tensor.matmul(z[:, :], cmid_t[:, :], _fb[:, :],
                                           start=False, stop=False,
                                           skip_group_check=True)
                          nc.tensor.matmul(z[:, :], clo_t[:, :], _fb[:, :],
                                           start=False, stop=True,
                                           skip_group_check=True)
                          if 'z' in DUP:
                              nc.tensor.matmul(z[:, :], chi_t[:, :], _fb[:, :],
                                               start=True, stop=True)
                          zs[kk] = z
                      if 2 <= slot <= 65:
                          kk = slot - 2
                          sb_k = statb_t[:, 128 * kk:128 * (kk + 1)]
                          A1 = psA.tile([128, F], f32, tag="A1")
                          nc.tensor.matmul(A1[:, :], sb_k, af16[:, :],
                                           start=True, stop=True)
                          A1s[kk] = A1
                          zS = sb.tile([128, F], f32, tag="zS")
                          _z = zs.pop(kk)
                          nc.scalar.copy(zS[:, :], _z[:, :])
                          if 'zS' in DUP:
                              nc.scalar.copy(zS[:, :], _z[:, :])
                          zSs[kk] = zS
                      if 3 <= slot <= 66:
                          kk = slot - 3
                          P = sb.tile([128, F], f32, tag="P")
                          _a1 = A1s.pop(kk); _zs = zSs.pop(kk)
                          nc.vector.scalar_tensor_tensor(P[:, :], _a1[:, :],
                                                         iota_t[:, :], _zs[:, :],
                                                         Alu.is_equal, Alu.mult)
                          if 'P' in DUP:
                              nc.vector.scalar_tensor_tensor(P[:, :], _a1[:, :],
                                                             iota_t[:, :], _zs[:, :],
                                                             Alu.is_equal, Alu.mult)
                          Ps[kk] = P
                      if slot >= 4:
                          kk = slot - 4
                          sr_k = statr_t[:, 128 * kk:128 * (kk + 1)]
                          nc.tensor.matmul(Yt[:, :], sr_k, Ps.pop(kk)[:, :],
                                           start=(kk == 0), stop=(kk == 63),
                                           skip_group_check=True)
                YS = sb.tile([128, F], f32, tag="YS")
                nc.scalar.copy(YS[:, :], Yt[:, :])
                R = psB.tile([128, F], f32, tag="xT")
                for q in range(4):
                    nc.tensor.transpose(
                        R[:, 128 * q:128 * (q + 1)],
                        YS[:, 128 * q:128 * (q + 1)], eye_t[:, :])
                RS = sb.tile([128, F], f32, tag="RS")
                nc.scalar.copy(RS[:, :], R[:, :])
                nc.sync.dma_start(out=yap[:, bass.ts(t, F)], in_=RS[:, :])

            if n_tiles <= 4:
                for t in range(n_tiles):
                    body(t)
            else:
                tc.For_i_unrolled(0, n_tiles, 1, body, max_unroll=n_unroll)


def _build_bass_kernel2(cols: int):
    from concourse import mybir
    from concourse.tile import TileContext
    from concourse.bass2jax import bass_jit

    assert cols % F == 0
    f32 = mybir.dt.float32

    @bass_jit
    def k(nc, x, chi, cmid, clo, statb, statr, iotap, eye):
        y = nc.dram_tensor("y", [128, cols], f32, kind="ExternalOutput")
        with TileContext(nc) as tc:
            _emit(nc, tc, x.ap(), y.ap(), cols,
                  ((chi.ap(), cmid.ap(), clo.ap()), statb.ap(), statr.ap(),
                   iotap.ap(), eye.ap()))
        return y

    return k


def _build_timing_kernel(cols: int, pair_reps: int = 1):
    """Same device work; y internal, tiny external output."""
    from concourse import mybir
    from concourse.tile import TileContext
    from concourse.bass2jax import bass_jit

    f32 = mybir.dt.float32

    @bass_jit
    def k(nc, x, chi, cmid, clo, statb, statr, iotap, eye):
        y = nc.dram_tensor("y_int", [128, cols], f32)
        out = nc.dram_tensor("out", [128, 8], f32, kind="ExternalOutput")
        with TileContext(nc) as tc:
            _emit(nc, tc, x.ap(), y.ap(), cols,
                  ((chi.ap(), cmid.ap(), clo.ap()), statb.ap(), statr.ap(),
                   iotap.ap(), eye.ap()),
                  pair_reps=pair_reps)
            with tc.tile_pool(name="fin", bufs=1) as fin:
                o = fin.tile([128, 8], f32)
                nc.sync.dma_start(out=o[:, :], in_=y.ap()[:, 0:8])
                nc.sync.dma_start(out=out.ap()[:, :], in_=o[:, :])
        return out

    return k


def _build_trivial_kernel():
    from concourse import mybir
    from concourse.tile import TileContext
    from concourse.bass2jax import bass_jit

    f32 = mybir.dt.float32

    @bass_jit
    def k(nc, small):
        out = nc.dram_tensor("out", [128, 8], f32, kind="ExternalOutput")
        with TileContext(nc) as tc:
            with tc.tile_pool(name="sb", bufs=1) as sb:
                o = sb.tile([128, 8], f32)
                nc.sync.dma_start(out=o[:, :], in_=small.ap()[:, :])
                nc.sync.dma_start(out=out.ap()[:, :], in_=o[:, :])
        return out

    return k


def kernel(x, sorted_values, cdf_table, scale):
    import jax

    x = np.asarray(x)
    out_dtype = x.dtype
    orig_shape = x.shape
    total = x.size
    assert total % (N_CORES * 128) == 0
    cols = total // (N_CORES * 128)

    tbl = _build_device_table(np.asarray(sorted_values), np.asarray(cdf_table),
                              np.asarray(scale))
    consts = _consts(tbl)

    if cols not in _COMPILED:
        _COMPILED[cols] = _build_bass_kernel2(cols)
    k = _COMPILED[cols]

    devices = jax.devices()[:N_CORES]
    x_shards = x.reshape(N_CORES, 128, cols)
    flat_consts = consts[0] + consts[1:]
    outs = []
    for i, dev in enumerate(devices):
        args = [jax.device_put(a, dev) for a in (x_shards[i],) + flat_consts]
        outs.append(k(*args))
    res = [np.asarray(o) for o in outs]
    return np.stack(res, axis=0).reshape(orig_shape).astype(out_dtype, copy=False)


def measure_device_time_ns(inputs, n_rep: int = 5):
    import jax, time

    x = np.asarray(inputs["x"])
    cols = x.size // (N_CORES * 128)
    tbl = _build_device_table(np.asarray(inputs["sorted_values"]),
                              np.asarray(inputs["cdf_table"]),
                              np.asarray(inputs["scale"]))
    consts = _consts(tbl)
    dev = jax.devices()[0]
    x0 = x.reshape(N_CORES, 128, cols)[0]
    args = [jax.device_put(a, dev) for a in (x0,) + consts[0] + consts[1:]]

    if cols not in _COMPILED_T:
        _COMPILED_T[cols] = _build_timing_kernel(cols)
    kt = _COMPILED_T[cols]
    o = kt(*args); jax.block_until_ready(o)
    ts = []
    for _ in range(n_rep):
        t0 = time.perf_counter()
        o = kt(*args)
        jax.block_until_ready(o)
        ts.append(time.perf_counter() - t0)
    t_kernel = min(ts)

    ktriv = _build_trivial_kernel()
    small = jax.device_put(np.zeros((128, 8), np.float32), dev)
    o = ktriv(small); jax.block_until_ready(o)
    ts = []
    for _ in range(n_rep):
        t0 = time.perf_counter()
        o = ktriv(small)
        jax.block_until_ready(o)
        ts.append(time.perf_counter() - t0)
    t_triv = min(ts)

    print(f"  timing-variant wall: {t_kernel*1e3:.2f} ms; "
          f"trivial-kernel wall: {t_triv*1e3:.2f} ms")
    return max(t_kernel - t_triv, 0.0) * 1e9

